# revision 1
# baseline (speedup 1.0000x reference)
"""DLRM (nn_DLRM_RPC) Trainium2 Bass kernel.

Strategy: pure data-parallel over batch across 8 NeuronCores; embedding
tables replicated in each core's HBM (966 MB bf16 total for all cores'
shares of device HBM - fits easily), so no collectives are needed.

Per core (2048 samples, 4 sample-tiles of 512):
  - one multi-index indirect DMA per 128-sample chunk gathers all 26
    embedding rows per sample (bf16, host-precast) into SBUF sample-major
  - PE transposes flip the gathered vectors feature-major into a grouped
    layout Eall[d, 123*g + 32*s + t]  (g = sample group of 4, t = slot:
    0 = bottom-MLP output x, 1..26 = embeddings)
  - bottom MLP runs feature-major and drops x straight into slot 0
  - per-group Gram matmuls B_g = blk^T @ blk give all 27x27 interaction
    dot products for 4 samples at once (diagonal s-blocks useful)
  - a partition-base-shifted copy pass rearranges Z into 7 K-stacked
    tiles Zstk_q[32u+j, b] = Z_b[4q+u, j]
  - top MLP consumes [x ; Zstk] with host-rearranged W0 (symmetric Z
    means only lower-triangle weights are placed), ReLU/Sigmoid fused
    into the PSUM->SBUF drains on the scalar engine.

All matmuls bf16 with fp32 PSUM accumulation.
"""

import os
import sys

import numpy as np

for _p in ("/opt/trn_rl_repo",):
    if _p not in sys.path and os.path.isdir(_p):
        sys.path.insert(0, _p)

import ml_dtypes

import concourse.bass as bass
import concourse.bacc as bacc
import concourse.mybir as mybir
import concourse.tile as tile
from concourse import bass_utils
from concourse.bass_interp import get_hw_module
from concourse.masks import make_identity

BF16 = ml_dtypes.bfloat16
F32 = np.float32

N_CORES = 8
B = 16384
SPC = B // N_CORES        # samples per core: 2048
NT = 27                   # slots: x + 26 tables
NE = 26
VOCAB = 50000
D = 128
BW = 123                  # group block width: 32*3 + 27
TS = 512                  # samples per tile
NTILES = SPC // TS        # 4
G = TS // 4               # groups per tile: 128
CH = TS // 128            # 128-sample chunks per tile: 4

LI, LJ = np.tril_indices(NT, -1)

_dt_bf16 = mybir.dt.bfloat16
_dt_f32 = mybir.dt.float32
_dt_i32 = mybir.dt.int32

_CACHE = {}


def _emit(tc, t):
    from contextlib import ExitStack

    nc = tc.nc
    Relu = mybir.ActivationFunctionType.Relu
    Sigmoid = mybir.ActivationFunctionType.Sigmoid

    with ExitStack() as ctx:
        sb = ctx.enter_context(tc.tile_pool(name="sb", bufs=1))
        db = ctx.enter_context(tc.tile_pool(name="db", bufs=2))
        mmps = ctx.enter_context(tc.tile_pool(name="mmps", bufs=2, space="PSUM"))
        grps = ctx.enter_context(tc.tile_pool(name="grps", bufs=2, space="PSUM"))
        trps = ctx.enter_context(tc.tile_pool(name="trps", bufs=2, space="PSUM"))
        w3ps = ctx.enter_context(tc.tile_pool(name="w3ps", bufs=1, space="PSUM"))

        ident = sb.tile([128, 128], _dt_bf16)
        make_identity(nc, ident[:])

        # --- load weights/inputs that stay resident ---
        def load(name, shape, dtype=_dt_bf16):
            tl = sb.tile(shape, dtype, name=name)
            nc.sync.dma_start(tl[:], t[name][:])
            return tl

        dxt = load("dxt", [16, SPC])
        bw0 = load("bw0", [16, 512])
        bb0 = load("bb0", [128, 4], _dt_f32)
        bw1 = load("bw1", [128, 4 * 256])
        bb1 = load("bb1", [128, 2], _dt_f32)
        bw2 = load("bw2", [128, 2 * 128])
        bb2 = load("bb2", [128, 1], _dt_f32)
        w0x = load("w0x", [128, 1024])
        wz = load("wz", [128, 7 * 1024])
        tb0 = load("tb0", [128, 8], _dt_f32)
        w1 = load("w1", [128, 8 * 1024])
        tb1 = load("tb1", [128, 8], _dt_f32)
        w2 = load("w2", [128, 8 * 512])
        tb2 = load("tb2", [128, 4], _dt_f32)
        w3 = load("w3", [128, 4])
        tb3 = load("tb3", [1, 1], _dt_f32)

        eall = sb.tile([128, BW * G], _dt_bf16)
        zsb = sb.tile([128, BW * G], _dt_bf16)
        zstk = [sb.tile([128, TS], _dt_bf16, name=f"zstk{q}") for q in range(7)]
        for q in range(7):
            nc.vector.memset(zstk[q][:], 0.0)

        eb = eall[:]
        pstep = eb.ap[0]
        # zero the 5 pad columns after each of the first 3 s-blocks
        pad_ap = bass.AP(eb.tensor, eb.offset + 27,
                         [pstep, [BW, G], [32, 3], [1, 5]])
        nc.vector.memset(pad_ap, 0.0)

        zb = zsb[:]
        zsb3 = zb.rearrange("p (g c) -> p g c", c=BW)

        for n in range(NTILES):
            # ---- Phase A: gather + transpose into Eall ----
            for c in range(CH):
                C = CH * n + c
                idxt = db.tile([128, NE], _dt_i32, name="idxt")
                nc.sync.dma_start(idxt[:], t["idx"][128 * C:128 * (C + 1), :])
                esm = db.tile([128, NE * D], _dt_bf16, name="esm")
                nc.gpsimd.indirect_dma_start(
                    out=esm[:], out_offset=None,
                    in_=t["tbl"][:],
                    in_offset=bass.IndirectOffsetOnAxis(ap=idxt[:], axis=0),
                )
                for t8 in range(4):
                    nt8 = 8 if t8 < 3 else 2
                    trp = trps.tile([128, 128 * nt8], _dt_bf16,
                                    name="trp", tag="trp")
                    for k in range(nt8):
                        ti = 8 * t8 + k
                        nc.tensor.transpose(
                            trp[:, 128 * k:128 * (k + 1)],
                            esm[:, 128 * ti:128 * (ti + 1)], ident[:])
                    dst = bass.AP(
                        eb.tensor, eb.offset + BW * 32 * c + 8 * t8 + 1,
                        [pstep, [1, nt8], [BW, 32], [32, 4]])
                    nc.vector.tensor_copy(dst, trp[:])

            # ---- Phase B: bottom MLP -> x into Eall slot 0 ----
            h0 = db.tile([128, 4 * 512], _dt_bf16, name="h0")
            for m in range(4):
                ps = mmps.tile([128, 512], _dt_f32, name="mm", tag="mm")
                nc.tensor.matmul(ps[:], bw0[:, 128 * m:128 * (m + 1)],
                                 dxt[:, TS * n:TS * (n + 1)],
                                 start=True, stop=True)
                nc.scalar.activation(h0[:, 512 * m:512 * (m + 1)], ps[:],
                                     Relu, bias=bb0[:, m:m + 1])
            h1b = db.tile([128, 2 * 512], _dt_bf16, name="h1b")
            for m in range(2):
                ps = mmps.tile([128, 512], _dt_f32, name="mm", tag="mm")
                for k in range(4):
                    nc.tensor.matmul(
                        ps[:], bw1[:, 256 * k + 128 * m:256 * k + 128 * (m + 1)],
                        h0[:, 512 * k:512 * (k + 1)],
                        start=(k == 0), stop=(k == 3))
                nc.scalar.activation(h1b[:, 512 * m:512 * (m + 1)], ps[:],
                                     Relu, bias=bb1[:, m:m + 1])
            ps = mmps.tile([128, 512], _dt_f32, name="mm", tag="mm")
            for k in range(2):
                nc.tensor.matmul(ps[:], bw2[:, 128 * k:128 * (k + 1)],
                                 h1b[:, 512 * k:512 * (k + 1)],
                                 start=(k == 0), stop=(k == 1))
            xdst = bass.AP(eb.tensor, eb.offset, [pstep, [BW, G], [32, 4]])
            nc.scalar.activation(xdst, ps[:], Relu, bias=bb2[:, 0:1])

            # ---- Phase C: Gram matmuls ----
            for r in range(G // 4):
                bank = grps.tile([128, 4 * BW], _dt_f32, name="grb", tag="gr")
                for k in range(4):
                    g = 4 * r + k
                    blk = eall[:, BW * g:BW * (g + 1)]
                    nc.tensor.matmul(bank[0:BW, BW * k:BW * (k + 1)],
                                     blk, blk, start=True, stop=True)
                nc.vector.tensor_copy(zsb[0:BW, 4 * BW * r:4 * BW * (r + 1)],
                                      bank[0:BW, :])

            # ---- Phase D: scramble Z into K-stacked tiles ----
            for i in range(NT):
                q, u = i // 4, i % 4
                for s in range(4):
                    src = zsb3[32 * s:32 * s + 27, :, 32 * s + i]
                    dst = zstk[q][:].rearrange("p (g s) -> p g s", s=4)[
                        32 * u:32 * u + 27, :, s]
                    nc.vector.tensor_copy(dst, src)

            # ---- Phase E: top MLP ----
            xap = bass.AP(eb.tensor, eb.offset, [pstep, [BW, G], [32, 4]])
            h1t = db.tile([128, 8 * 512], _dt_bf16, name="h1t")
            for m in range(8):
                ps = mmps.tile([128, 512], _dt_f32, name="mm", tag="mm")
                nc.tensor.matmul(ps[:], w0x[:, 128 * m:128 * (m + 1)], xap,
                                 start=True, stop=False)
                for q in range(7):
                    nc.tensor.matmul(
                        ps[:], wz[:, 1024 * q + 128 * m:1024 * q + 128 * (m + 1)],
                        zstk[q][:], start=False, stop=(q == 6))
                nc.scalar.activation(h1t[:, 512 * m:512 * (m + 1)], ps[:],
                                     Relu, bias=tb0[:, m:m + 1])
            h2t = db.tile([128, 8 * 512], _dt_bf16, name="h2t")
            for m in range(8):
                ps = mmps.tile([128, 512], _dt_f32, name="mm", tag="mm")
                for k in range(8):
                    nc.tensor.matmul(
                        ps[:], w1[:, 1024 * k + 128 * m:1024 * k + 128 * (m + 1)],
                        h1t[:, 512 * k:512 * (k + 1)],
                        start=(k == 0), stop=(k == 7))
                nc.scalar.activation(h2t[:, 512 * m:512 * (m + 1)], ps[:],
                                     Relu, bias=tb1[:, m:m + 1])
            h3t = db.tile([128, 4 * 512], _dt_bf16, name="h3t")
            for m in range(4):
                ps = mmps.tile([128, 512], _dt_f32, name="mm", tag="mm")
                for k in range(8):
                    nc.tensor.matmul(
                        ps[:], w2[:, 512 * k + 128 * m:512 * k + 128 * (m + 1)],
                        h2t[:, 512 * k:512 * (k + 1)],
                        start=(k == 0), stop=(k == 7))
                nc.scalar.activation(h3t[:, 512 * m:512 * (m + 1)], ps[:],
                                     Relu, bias=tb2[:, m:m + 1])
            ps3 = w3ps.tile([1, 512], _dt_f32, name="w3p", tag="w3")
            for k in range(4):
                nc.tensor.matmul(ps3[:], w3[:, k:k + 1],
                                 h3t[:, 512 * k:512 * (k + 1)],
                                 start=(k == 0), stop=(k == 3))
            outsb = db.tile([1, 512], _dt_f32, name="outsb")
            nc.scalar.activation(outsb[:], ps3[:], Sigmoid, bias=tb3[0:1, 0:1])
            nc.sync.dma_start(t["out"][n:n + 1, :], outsb[:])


def _build():
    if "nc" in _CACHE:
        return _CACHE["nc"]
    nc = bacc.Bacc("TRN2", target_bir_lowering=False, debug=False,
                   num_devices=N_CORES)
    t = {}

    def dram(name, shape, dt, kind="ExternalInput"):
        t[name] = nc.dram_tensor(name, shape, dt, kind=kind).ap()

    dram("tbl", [NE * VOCAB, D], _dt_bf16)
    dram("idx", [SPC, NE], _dt_i32)
    dram("dxt", [16, SPC], _dt_bf16)
    dram("bw0", [16, 512], _dt_bf16)
    dram("bb0", [128, 4], _dt_f32)
    dram("bw1", [128, 4 * 256], _dt_bf16)
    dram("bb1", [128, 2], _dt_f32)
    dram("bw2", [128, 2 * 128], _dt_bf16)
    dram("bb2", [128, 1], _dt_f32)
    dram("w0x", [128, 1024], _dt_bf16)
    dram("wz", [128, 7 * 1024], _dt_bf16)
    dram("tb0", [128, 8], _dt_f32)
    dram("w1", [128, 8 * 1024], _dt_bf16)
    dram("tb1", [128, 8], _dt_f32)
    dram("w2", [128, 8 * 512], _dt_bf16)
    dram("tb2", [128, 4], _dt_f32)
    dram("w3", [128, 4], _dt_bf16)
    dram("tb3", [1, 1], _dt_f32)
    dram("out", [NTILES, TS], _dt_f32, kind="ExternalOutput")

    with tile.TileContext(nc) as tc:
        _emit(tc, t)
    nc.compile()

    _CACHE["nc"] = nc
    return nc


def _ktile(w, kt, m):
    """[K, M] -> [128, (K//128) * M] with column kt*M + mm = w[128*kt + p, mm]."""
    K, Mo = w.shape
    return np.ascontiguousarray(
        w.reshape(K // 128, 128, Mo).transpose(1, 0, 2).reshape(128, -1))


def _shared_inputs(inputs):
    emb = np.asarray(inputs["emb_tables"])
    tbl = np.ascontiguousarray(
        emb.astype(BF16).reshape(NE * VOCAB, D))

    sh = {"tbl": tbl}
    sh["bw0"] = np.zeros((16, 512), BF16)
    sh["bw0"][:13] = np.asarray(inputs["bot_W0"]).astype(BF16)
    sh["bb0"] = np.asarray(inputs["bot_b0"]).astype(F32).reshape(4, 128).T.copy()
    sh["bw1"] = _ktile(np.asarray(inputs["bot_W1"]).astype(BF16), 4, 256)
    sh["bb1"] = np.asarray(inputs["bot_b1"]).astype(F32).reshape(2, 128).T.copy()
    sh["bw2"] = _ktile(np.asarray(inputs["bot_W2"]).astype(BF16), 2, 128)
    sh["bb2"] = np.asarray(inputs["bot_b2"]).astype(F32).reshape(1, 128).T.copy()

    w0 = np.asarray(inputs["top_W0"]).astype(F32)
    sh["w0x"] = w0[:128].astype(BF16)
    wgrid = np.zeros((NT, NT, 1024), F32)
    wgrid[LI, LJ] = w0[128:479]
    wz4 = np.zeros((7, 128, 1024), F32)
    for i in range(NT):
        q, u = i // 4, i % 4
        wz4[q, 32 * u:32 * u + NT] = wgrid[i]
    sh["wz"] = np.ascontiguousarray(
        wz4.transpose(1, 0, 2).reshape(128, 7 * 1024)).astype(BF16)
    sh["tb0"] = np.asarray(inputs["top_b0"]).astype(F32).reshape(8, 128).T.copy()
    sh["w1"] = _ktile(np.asarray(inputs["top_W1"]).astype(BF16), 8, 1024)
    sh["tb1"] = np.asarray(inputs["top_b1"]).astype(F32).reshape(8, 128).T.copy()
    sh["w2"] = _ktile(np.asarray(inputs["top_W2"]).astype(BF16), 8, 512)
    sh["tb2"] = np.asarray(inputs["top_b2"]).astype(F32).reshape(4, 128).T.copy()
    sh["w3"] = _ktile(np.asarray(inputs["top_W3"]).astype(BF16), 4, 1)
    sh["tb3"] = np.asarray(inputs["top_b3"]).astype(F32).reshape(1, 1)
    return sh


def _in_maps(inputs):
    sh = _shared_inputs(inputs)
    idx = np.asarray(inputs["indices"]).astype(np.int64)      # [26, B]
    gidx = (idx + (np.arange(NE) * VOCAB)[:, None]).astype(np.int32)
    dx = np.asarray(inputs["dense_x"]).astype(F32)            # [B, 13]
    maps = []
    for core in range(N_CORES):
        sl = slice(SPC * core, SPC * (core + 1))
        m = dict(sh)
        m["idx"] = np.ascontiguousarray(gidx[:, sl].T)        # [2048, 26]
        dxt = np.zeros((16, SPC), BF16)
        dxt[:13] = dx[sl].T.astype(BF16)
        m["dxt"] = dxt
        maps.append(m)
    return maps


def _run(inputs, trace=False):
    nc = _build()
    maps = _in_maps(inputs)
    old_m = nc.m
    nc.m = _CACHE.setdefault("hwm", get_hw_module(nc.m))
    try:
        res = bass_utils.run_bass_kernel_spmd(
            nc, maps, core_ids=list(range(N_CORES)), trace=trace)
    finally:
        nc.m = old_m
    out = np.concatenate([r["out"].reshape(-1) for r in res.results])
    return out.astype(F32).reshape(B, 1), res


def kernel(**inputs):
    out, _ = _run(inputs, trace=False)
    return out



# revision 2
# speedup vs baseline: 1.0059x; 1.0059x over previous
"""DLRM (nn_DLRM_RPC) Trainium2 Bass kernel (optimized).

Strategy: pure data-parallel over batch across 8 NeuronCores; embedding
tables replicated in each core's HBM (bf16), no collectives.

Changes vs v1 baseline:
  - group block width 123 -> 128 (4 s-blocks of 32), so every Gram
    stationary operand is a full 128-column bf16 load (FWL-eligible)
  - PE transposes done as regular matmuls (lhsT = gathered data,
    rhs = identity): stationary load IS the transpose, FWL applies,
    and the PE HAM clock-gate stays warm (transpose-mode doesn't
    count as PE-busy at full rate)
  - bottom MLP emitted first, Gram interleaved per gather-chunk
    (shorter per-tile pipeline fill), Gram PSUM drained on the
    scalar engine (relieves the DVE for the Z scatter)
  - software-pipelined emission: tile n's top MLP is emitted after
    tile n+1's gather/Gram, with a double-buffered feature buffer,
    so the PE stays busy while the DVE scatters Z
  - deeper (3-buffer) prefetch on the indirect-gather chunks; PE
    warm-up matmuls under the initial DMA shadow
  (Z stays in the 32-aligned 7-K-tile strip layout: engine SBUF
   access patterns may only start at partition 0/32/64/96, which
   rules out denser packing of the 351 triangle entries.)

Per core (2048 samples, 4 sample-tiles of 512):
  Phase B: bottom MLP feature-major, x lands in slot 0 of Eall
  Phase A: indirect-DMA gather 26 rows/sample (bf16) + PE transpose
           into grouped layout Eall[d, 128*g + 32*s + t]
           (g = group of 4 samples, s = sample-in-group, t = slot:
            0 = bottom-MLP x, 1..26 = embeddings, 27..31 = zero pad)
           + per-chunk Gram B_g = blk^T blk (27x27 dots, 4 samples)
  Phase D: scatter lower-triangle Z rows into 7 K-tiles (32-strips)
  Phase E: top MLP, ReLU/Sigmoid fused in PSUM drains
"""

import os
import sys

import numpy as np

for _p in ("/opt/trn_rl_repo",):
    if _p not in sys.path and os.path.isdir(_p):
        sys.path.insert(0, _p)

import ml_dtypes

import concourse.bass as bass
import concourse.bacc as bacc
import concourse.mybir as mybir
import concourse.tile as tile
from concourse import bass_utils
from concourse.bass_interp import get_hw_module
from concourse.masks import make_identity

BF16 = ml_dtypes.bfloat16
F32 = np.float32

N_CORES = 8
B = 16384
SPC = B // N_CORES        # samples per core: 2048
NT = 27                   # slots: x + 26 tables
NE = 26
VOCAB = 50000
D = 128
BW = 128                  # group block width: 4 s-blocks of 32
TS = 512                  # samples per tile
NTILES = SPC // TS        # 4
G = TS // 4               # groups per tile: 128
CH = TS // 128            # 128-sample chunks per tile: 4

LI, LJ = np.tril_indices(NT, -1)

NZK = 7                   # zstk K-tiles: Z row i=4q+u at zstk[q][32u+j]

_dt_bf16 = mybir.dt.bfloat16
_dt_f32 = mybir.dt.float32
_dt_i32 = mybir.dt.int32

_CACHE = {}


def _emit(tc, t):
    from contextlib import ExitStack

    nc = tc.nc
    Relu = mybir.ActivationFunctionType.Relu
    Sigmoid = mybir.ActivationFunctionType.Sigmoid

    with ExitStack() as ctx:
        sb = ctx.enter_context(tc.tile_pool(name="sb", bufs=1))
        gp = ctx.enter_context(tc.tile_pool(name="gp", bufs=3))
        hp = ctx.enter_context(tc.tile_pool(name="hp", bufs=1))
        op = ctx.enter_context(tc.tile_pool(name="op", bufs=2))
        mmps = ctx.enter_context(tc.tile_pool(name="mmps", bufs=2, space="PSUM"))
        grps = ctx.enter_context(tc.tile_pool(name="grps", bufs=3, space="PSUM"))
        trps = ctx.enter_context(tc.tile_pool(name="trps", bufs=2, space="PSUM"))
        w3ps = ctx.enter_context(tc.tile_pool(name="w3ps", bufs=1, space="PSUM"))

        ident = sb.tile([128, 128], _dt_bf16)
        make_identity(nc, ident[:])

        # warm the PE HAM clock-gate during the initial DMA wait
        warm = mmps.tile([128, 512], _dt_f32, name="mm", tag="mm")
        for _ in range(24):
            nc.tensor.matmul(warm[:, 0:128], ident[:], ident[:],
                             start=True, stop=True)

        # --- load weights/inputs that stay resident ---
        def load(name, shape, dtype=_dt_bf16):
            tl = sb.tile(shape, dtype, name=name)
            nc.sync.dma_start(tl[:], t[name][:])
            return tl

        dxt = load("dxt", [16, SPC])
        bw0 = load("bw0", [16, 512])
        bb0 = load("bb0", [128, 4], _dt_f32)
        bw1 = load("bw1", [128, 4 * 256])
        bb1 = load("bb1", [128, 2], _dt_f32)
        bw2 = load("bw2", [128, 2 * 128])
        bb2 = load("bb2", [128, 1], _dt_f32)
        w0x = load("w0x", [128, 1024])
        wzd = load("wzd", [128, NZK * 1024])
        tb0 = load("tb0", [128, 8], _dt_f32)
        w1 = load("w1", [128, 8 * 1024])
        tb1 = load("tb1", [128, 8], _dt_f32)
        w2 = load("w2", [128, 8 * 512])
        tb2 = load("tb2", [128, 4], _dt_f32)
        w3 = load("w3", [128, 4])
        tb3 = load("tb3", [1, 1], _dt_f32)

        ealls = [sb.tile([128, BW * G], _dt_bf16, name=f"eall{v}")
                 for v in range(2)]
        zsb = sb.tile([128, BW * G], _dt_bf16)
        zpk = [sb.tile([128, TS], _dt_bf16, name=f"zpk{q}") for q in range(NZK)]
        # rows j >= i of each strip are never written by the scatter;
        # zero all strips once (their weights are zero).
        for q in range(NZK):
            nc.vector.memset(zpk[q][:], 0.0)

        for v in range(2):
            eb = ealls[v][:]
            pstep = eb.ap[0]
            # zero the 5 pad slots after slot 26 in each 32-wide s-block
            pad_ap = bass.AP(eb.tensor, eb.offset + 27,
                             [pstep, [BW, G], [32, 4], [1, 5]])
            nc.vector.memset(pad_ap, 0.0)

        zb = zsb[:]
        zsb3 = zb.rearrange("p (g c) -> p g c", c=BW)

        def phase_a_chunk(n, c, eb, pstep, eall):
            # gather + transpose one 128-sample chunk into Eall, then
            # immediately run that chunk's 32 Gram groups (keeps the PE
            # fed and shortens the per-tile pipeline fill).
            C = CH * n + c
            idxt = gp.tile([128, NE], _dt_i32, name="idxt")
            nc.sync.dma_start(idxt[:], t["idx"][128 * C:128 * (C + 1), :])
            esm = gp.tile([128, NE * D], _dt_bf16, name="esm")
            nc.gpsimd.indirect_dma_start(
                out=esm[:], out_offset=None,
                in_=t["tbl"][:],
                in_offset=bass.IndirectOffsetOnAxis(ap=idxt[:], axis=0),
            )
            for t8 in range(7):
                nt8 = 4 if t8 < 6 else 2
                trp = trps.tile([128, 128 * nt8], _dt_f32,
                                name="trp", tag="trp")
                for k in range(nt8):
                    ti = 4 * t8 + k
                    # out = esm_slice.T @ I : the stationary load is
                    # the transpose; FWL applies (128-col bf16).
                    nc.tensor.matmul(
                        trp[:, 128 * k:128 * (k + 1)],
                        esm[:, 128 * ti:128 * (ti + 1)], ident[:],
                        start=True, stop=True)
                tb = trp[:]
                # iterate (group, s, slot) so the nt8 adjacent slots are
                # innermost: dst runs of nt8 contiguous bf16 elements
                # instead of isolated 2-byte writes.
                srcv = bass.AP(tb.tensor, tb.offset,
                               [tb.ap[0], [4, 32], [1, 4], [128, nt8]])
                dst = bass.AP(
                    eb.tensor, eb.offset + BW * 32 * c + 4 * t8 + 1,
                    [pstep, [BW, 32], [32, 4], [1, nt8]])
                nc.vector.tensor_copy(dst, srcv)
            # Gram for this chunk's 32 groups
            for r in range(8):
                bank = grps.tile([128, 4 * BW], _dt_f32, name="grb", tag="gr")
                for k in range(4):
                    g = 32 * c + 4 * r + k
                    blk = eall[:, BW * g:BW * (g + 1)]
                    nc.tensor.matmul(bank[:, BW * k:BW * (k + 1)],
                                     blk, blk, start=True, stop=True)
                ro = 32 * c + 4 * r
                nc.scalar.activation(
                    zsb[:, BW * ro:BW * (ro + 4)], bank[:],
                    mybir.ActivationFunctionType.Copy)

        def phase_b(n, eb, pstep):
            # bottom MLP -> x into Eall slot 0
            h0 = hp.tile([128, 4 * 512], _dt_bf16, name="h0")
            for m in range(4):
                ps = mmps.tile([128, 512], _dt_f32, name="mm", tag="mm")
                nc.tensor.matmul(ps[:], bw0[:, 128 * m:128 * (m + 1)],
                                 dxt[:, TS * n:TS * (n + 1)],
                                 start=True, stop=True)
                nc.scalar.activation(h0[:, 512 * m:512 * (m + 1)], ps[:],
                                     Relu, bias=bb0[:, m:m + 1])
            h1b = hp.tile([128, 2 * 512], _dt_bf16, name="h1b")
            for m in range(2):
                ps = mmps.tile([128, 512], _dt_f32, name="mm", tag="mm")
                for k in range(4):
                    nc.tensor.matmul(
                        ps[:], bw1[:, 256 * k + 128 * m:256 * k + 128 * (m + 1)],
                        h0[:, 512 * k:512 * (k + 1)],
                        start=(k == 0), stop=(k == 3))
                nc.scalar.activation(h1b[:, 512 * m:512 * (m + 1)], ps[:],
                                     Relu, bias=bb1[:, m:m + 1])
            ps = mmps.tile([128, 512], _dt_f32, name="mm", tag="mm")
            for k in range(2):
                nc.tensor.matmul(ps[:], bw2[:, 128 * k:128 * (k + 1)],
                                 h1b[:, 512 * k:512 * (k + 1)],
                                 start=(k == 0), stop=(k == 1))
            xdst = bass.AP(eb.tensor, eb.offset, [pstep, [BW, G], [32, 4]])
            nc.scalar.activation(xdst, ps[:], Relu, bias=bb2[:, 0:1])

        def phase_d(n):
            # scatter lower-triangle Z rows into 32-aligned strips.
            # zpk columns are s-major (col = 128*s + g  <->  sample 4g+s)
            # so each copy writes one contiguous 128-column run; odd rows
            # go to the scalar engine to split the load with the DVE.
            Copy = mybir.ActivationFunctionType.Copy
            for i in range(1, NT):
                q, u = i // 4, i % 4
                p0 = 32 * u
                for s in range(4):
                    src = zsb3[32 * s:32 * s + i, :, 32 * s + i]
                    dst = zpk[q][p0:p0 + i, 128 * s:128 * (s + 1)]
                    if i % 2:
                        nc.scalar.activation(dst, src, Copy)
                    else:
                        nc.vector.tensor_copy(dst, src)

        def phase_e(n, eb, pstep):
            # top MLP (layer 0: K = 128 x + 384 packed Z = 512)
            # s-major stream: element 128*s + g reads col 32*s + 128*g,
            # i.e. sample 4g+s -- matches the zpk column layout.
            xap = bass.AP(eb.tensor, eb.offset, [pstep, [32, 4], [BW, G]])
            h1t = hp.tile([128, 8 * 512], _dt_bf16, name="h1t")
            for m in range(8):
                ps = mmps.tile([128, 512], _dt_f32, name="mm", tag="mm")
                nc.tensor.matmul(ps[:], w0x[:, 128 * m:128 * (m + 1)], xap,
                                 start=True, stop=False)
                for q in range(NZK):
                    nc.tensor.matmul(
                        ps[:],
                        wzd[:, 1024 * q + 128 * m:1024 * q + 128 * (m + 1)],
                        zpk[q][:], start=False, stop=(q == NZK - 1))
                nc.scalar.activation(h1t[:, 512 * m:512 * (m + 1)], ps[:],
                                     Relu, bias=tb0[:, m:m + 1])
            h2t = hp.tile([128, 8 * 512], _dt_bf16, name="h2t")
            for m in range(8):
                ps = mmps.tile([128, 512], _dt_f32, name="mm", tag="mm")
                for k in range(8):
                    nc.tensor.matmul(
                        ps[:], w1[:, 1024 * k + 128 * m:1024 * k + 128 * (m + 1)],
                        h1t[:, 512 * k:512 * (k + 1)],
                        start=(k == 0), stop=(k == 7))
                nc.scalar.activation(h2t[:, 512 * m:512 * (m + 1)], ps[:],
                                     Relu, bias=tb1[:, m:m + 1])
            h3t = hp.tile([128, 4 * 512], _dt_bf16, name="h3t")
            for m in range(4):
                ps = mmps.tile([128, 512], _dt_f32, name="mm", tag="mm")
                for k in range(8):
                    nc.tensor.matmul(
                        ps[:], w2[:, 512 * k + 128 * m:512 * k + 128 * (m + 1)],
                        h2t[:, 512 * k:512 * (k + 1)],
                        start=(k == 0), stop=(k == 7))
                nc.scalar.activation(h3t[:, 512 * m:512 * (m + 1)], ps[:],
                                     Relu, bias=tb2[:, m:m + 1])
            ps3 = w3ps.tile([1, 512], _dt_f32, name="w3p", tag="w3")
            for k in range(4):
                nc.tensor.matmul(ps3[:], w3[:, k:k + 1],
                                 h3t[:, 512 * k:512 * (k + 1)],
                                 start=(k == 0), stop=(k == 3))
            outsb = op.tile([1, 512], _dt_f32, name="outsb")
            nc.scalar.activation(outsb[:], ps3[:], Sigmoid, bias=tb3[0:1, 0:1])
            nc.sync.dma_start(t["out"][n:n + 1, :], outsb[:])

        # software-pipelined emission: tile n's top MLP is emitted after
        # tile n+1's gather/bottom/Gram so the PE has work while the DVE
        # scatters tile n's Z. zpk is single-buffered: D(n) is emitted
        # after E(n-1) (program order covers the WAR hazard).
        for n in range(NTILES):
            eall = ealls[n % 2]
            eb = eall[:]
            pstep = eb.ap[0]
            phase_b(n, eb, pstep)
            for c in range(CH):
                phase_a_chunk(n, c, eb, pstep, eall)
            if n > 0:
                ep = ealls[(n - 1) % 2][:]
                phase_e(n - 1, ep, ep.ap[0])
            phase_d(n)
        ep = ealls[(NTILES - 1) % 2][:]
        phase_e(NTILES - 1, ep, ep.ap[0])


def _build():
    if "nc" in _CACHE:
        return _CACHE["nc"]
    nc = bacc.Bacc("TRN2", target_bir_lowering=False, debug=False,
                   num_devices=N_CORES)
    t = {}

    def dram(name, shape, dt, kind="ExternalInput"):
        t[name] = nc.dram_tensor(name, shape, dt, kind=kind).ap()

    dram("tbl", [NE * VOCAB, D], _dt_bf16)
    dram("idx", [SPC, NE], _dt_i32)
    dram("dxt", [16, SPC], _dt_bf16)
    dram("bw0", [16, 512], _dt_bf16)
    dram("bb0", [128, 4], _dt_f32)
    dram("bw1", [128, 4 * 256], _dt_bf16)
    dram("bb1", [128, 2], _dt_f32)
    dram("bw2", [128, 2 * 128], _dt_bf16)
    dram("bb2", [128, 1], _dt_f32)
    dram("w0x", [128, 1024], _dt_bf16)
    dram("wzd", [128, NZK * 1024], _dt_bf16)
    dram("tb0", [128, 8], _dt_f32)
    dram("w1", [128, 8 * 1024], _dt_bf16)
    dram("tb1", [128, 8], _dt_f32)
    dram("w2", [128, 8 * 512], _dt_bf16)
    dram("tb2", [128, 4], _dt_f32)
    dram("w3", [128, 4], _dt_bf16)
    dram("tb3", [1, 1], _dt_f32)
    dram("out", [NTILES, TS], _dt_f32, kind="ExternalOutput")

    with tile.TileContext(nc) as tc:
        _emit(tc, t)
    nc.compile()

    _CACHE["nc"] = nc
    return nc


def _ktile(w, kt, m):
    """[K, M] -> [128, (K//128) * M] with column kt*M + mm = w[128*kt + p, mm]."""
    K, Mo = w.shape
    return np.ascontiguousarray(
        w.reshape(K // 128, 128, Mo).transpose(1, 0, 2).reshape(128, -1))


def _shared_inputs(inputs):
    emb = np.asarray(inputs["emb_tables"])
    tbl = np.ascontiguousarray(
        emb.astype(BF16).reshape(NE * VOCAB, D))

    sh = {"tbl": tbl}
    sh["bw0"] = np.zeros((16, 512), BF16)
    sh["bw0"][:13] = np.asarray(inputs["bot_W0"]).astype(BF16)
    sh["bb0"] = np.asarray(inputs["bot_b0"]).astype(F32).reshape(4, 128).T.copy()
    sh["bw1"] = _ktile(np.asarray(inputs["bot_W1"]).astype(BF16), 4, 256)
    sh["bb1"] = np.asarray(inputs["bot_b1"]).astype(F32).reshape(2, 128).T.copy()
    sh["bw2"] = _ktile(np.asarray(inputs["bot_W2"]).astype(BF16), 2, 128)
    sh["bb2"] = np.asarray(inputs["bot_b2"]).astype(F32).reshape(1, 128).T.copy()

    w0 = np.asarray(inputs["top_W0"]).astype(F32)
    sh["w0x"] = w0[:128].astype(BF16)
    wzd_full = np.zeros((NZK * 128, 1024), F32)
    for f, (i, j) in enumerate(zip(LI, LJ)):
        q, u = i // 4, i % 4
        wzd_full[128 * q + 32 * u + j] = w0[128 + f]
    sh["wzd"] = _ktile(wzd_full, NZK, 1024).astype(BF16)
    sh["tb0"] = np.asarray(inputs["top_b0"]).astype(F32).reshape(8, 128).T.copy()
    sh["w1"] = _ktile(np.asarray(inputs["top_W1"]).astype(BF16), 8, 1024)
    sh["tb1"] = np.asarray(inputs["top_b1"]).astype(F32).reshape(8, 128).T.copy()
    sh["w2"] = _ktile(np.asarray(inputs["top_W2"]).astype(BF16), 8, 512)
    sh["tb2"] = np.asarray(inputs["top_b2"]).astype(F32).reshape(4, 128).T.copy()
    sh["w3"] = _ktile(np.asarray(inputs["top_W3"]).astype(BF16), 4, 1)
    sh["tb3"] = np.asarray(inputs["top_b3"]).astype(F32).reshape(1, 1)
    return sh


def _in_maps(inputs):
    sh = _shared_inputs(inputs)
    idx = np.asarray(inputs["indices"]).astype(np.int64)      # [26, B]
    gidx = (idx + (np.arange(NE) * VOCAB)[:, None]).astype(np.int32)
    dx = np.asarray(inputs["dense_x"]).astype(F32)            # [B, 13]
    maps = []
    for core in range(N_CORES):
        sl = slice(SPC * core, SPC * (core + 1))
        m = dict(sh)
        m["idx"] = np.ascontiguousarray(gidx[:, sl].T)        # [2048, 26]
        dxt = np.zeros((16, SPC), BF16)
        dxt[:13] = dx[sl].T.astype(BF16)
        m["dxt"] = dxt
        maps.append(m)
    return maps


def _run(inputs, trace=False):
    nc = _build()
    maps = _in_maps(inputs)
    old_m = nc.m
    nc.m = _CACHE.setdefault("hwm", get_hw_module(nc.m))
    try:
        res = bass_utils.run_bass_kernel_spmd(
            nc, maps, core_ids=list(range(N_CORES)), trace=trace)
    finally:
        nc.m = old_m
    outs = []
    for r in res.results:
        o = r["out"].reshape(NTILES, 4, 128)      # [n, s, g]
        outs.append(o.transpose(0, 2, 1).reshape(-1))  # sample 4g+s
    out = np.concatenate(outs)
    return out.astype(F32).reshape(B, 1), res


def kernel(**inputs):
    out, _ = _run(inputs, trace=False)
    return out


# revision 3
# speedup vs baseline: 1.0157x; 1.0097x over previous
"""DLRM (nn_DLRM_RPC) Trainium2 Bass kernel (optimized).

Strategy: pure data-parallel over batch across 8 NeuronCores; embedding
tables replicated in each core's HBM (bf16), no collectives.

Changes vs v1 baseline:
  - group block width 123 -> 128 (4 s-blocks of 32), so every Gram
    stationary operand is a full 128-column bf16 load (FWL-eligible)
  - PE transposes done as regular matmuls (lhsT = gathered data,
    rhs = identity): stationary load IS the transpose, FWL applies,
    and the PE HAM clock-gate stays warm (transpose-mode doesn't
    count as PE-busy at full rate)
  - bottom MLP emitted first, Gram interleaved per gather-chunk
    (shorter per-tile pipeline fill), Gram PSUM drained on the
    scalar engine (relieves the DVE for the Z scatter)
  - software-pipelined emission: tile n's top MLP is emitted after
    tile n+1's gather/Gram, with a double-buffered feature buffer,
    so the PE stays busy while the DVE scatters Z
  - deeper (3-buffer) prefetch on the indirect-gather chunks; PE
    warm-up matmuls under the initial DMA shadow
  (Z stays in the 32-aligned 7-K-tile strip layout: engine SBUF
   access patterns may only start at partition 0/32/64/96, which
   rules out denser packing of the 351 triangle entries.)

Per core (2048 samples, 4 sample-tiles of 512):
  Phase B: bottom MLP feature-major, x lands in slot 0 of Eall
  Phase A: indirect-DMA gather 26 rows/sample (bf16) + PE transpose
           into grouped layout Eall[d, 128*g + 32*s + t]
           (g = group of 4 samples, s = sample-in-group, t = slot:
            0 = bottom-MLP x, 1..26 = embeddings, 27..31 = zero pad)
           + per-chunk Gram B_g = blk^T blk (27x27 dots, 4 samples)
  Phase D: scatter lower-triangle Z rows into 7 K-tiles (32-strips)
  Phase E: top MLP, ReLU/Sigmoid fused in PSUM drains
"""

import os
import sys

import numpy as np

for _p in ("/opt/trn_rl_repo",):
    if _p not in sys.path and os.path.isdir(_p):
        sys.path.insert(0, _p)

import ml_dtypes

import concourse.bass as bass
import concourse.bacc as bacc
import concourse.mybir as mybir
import concourse.tile as tile
from concourse import bass_utils
from concourse.bass_interp import get_hw_module
from concourse.masks import make_identity

BF16 = ml_dtypes.bfloat16
F32 = np.float32

N_CORES = 8
B = 16384
SPC = B // N_CORES        # samples per core: 2048
NT = 27                   # slots: x + 26 tables
NE = 26
VOCAB = 50000
D = 128
BW = 128                  # group block width: 4 s-blocks of 32
TS = 512                  # samples per tile
NTILES = SPC // TS        # 4
G = TS // 4               # groups per tile: 128
CH = TS // 128            # 128-sample chunks per tile: 4

LI, LJ = np.tril_indices(NT, -1)

NZK = 7                   # zstk K-tiles: Z row i=4q+u at zstk[q][32u+j]

_dt_bf16 = mybir.dt.bfloat16
_dt_f32 = mybir.dt.float32
_dt_i32 = mybir.dt.int32

_CACHE = {}


def _emit(tc, t):
    from contextlib import ExitStack

    nc = tc.nc
    Relu = mybir.ActivationFunctionType.Relu
    Sigmoid = mybir.ActivationFunctionType.Sigmoid

    with ExitStack() as ctx:
        sb = ctx.enter_context(tc.tile_pool(name="sb", bufs=1))
        gp = ctx.enter_context(tc.tile_pool(name="gp", bufs=4))
        hp = ctx.enter_context(tc.tile_pool(name="hp", bufs=1))
        op = ctx.enter_context(tc.tile_pool(name="op", bufs=2))
        mmps = ctx.enter_context(tc.tile_pool(name="mmps", bufs=2, space="PSUM"))
        grps = ctx.enter_context(tc.tile_pool(name="grps", bufs=3, space="PSUM"))
        trps = ctx.enter_context(tc.tile_pool(name="trps", bufs=2, space="PSUM"))
        w3ps = ctx.enter_context(tc.tile_pool(name="w3ps", bufs=1, space="PSUM"))

        ident = sb.tile([128, 128], _dt_bf16)
        make_identity(nc, ident[:])

        # warm the PE HAM clock-gate during the initial DMA wait
        warm = mmps.tile([128, 512], _dt_f32, name="mm", tag="mm")
        for _ in range(24):
            nc.tensor.matmul(warm[:, 0:128], ident[:], ident[:],
                             start=True, stop=True)

        # --- load weights/inputs that stay resident ---
        def load(name, shape, dtype=_dt_bf16):
            tl = sb.tile(shape, dtype, name=name)
            nc.sync.dma_start(tl[:], t[name][:])
            return tl

        dxt = load("dxt", [16, SPC])
        bw0 = load("bw0", [16, 512])
        bb0 = load("bb0", [128, 4], _dt_f32)
        bw1 = load("bw1", [128, 4 * 256])
        bb1 = load("bb1", [128, 2], _dt_f32)
        bw2 = load("bw2", [128, 2 * 128])
        bb2 = load("bb2", [128, 1], _dt_f32)
        w0x = load("w0x", [128, 1024])
        wzd = load("wzd", [128, NZK * 1024])
        tb0 = load("tb0", [128, 8], _dt_f32)
        w1 = load("w1", [128, 8 * 1024])
        tb1 = load("tb1", [128, 8], _dt_f32)
        w2 = load("w2", [128, 8 * 512])
        tb2 = load("tb2", [128, 4], _dt_f32)
        w3 = load("w3", [128, 4])
        tb3 = load("tb3", [1, 1], _dt_f32)

        ealls = [sb.tile([128, BW * G], _dt_bf16, name=f"eall{v}")
                 for v in range(2)]
        zsb = sb.tile([128, BW * G], _dt_bf16)
        zpk = [sb.tile([128, TS], _dt_bf16, name=f"zpk{q}") for q in range(NZK)]
        # rows j >= i of each strip are never written by the scatter;
        # zero all strips once (their weights are zero).
        for q in range(NZK):
            nc.vector.memset(zpk[q][:], 0.0)

        for v in range(2):
            eb = ealls[v][:]
            pstep = eb.ap[0]
            # zero the 5 pad slots after slot 26 in each 32-wide s-block
            pad_ap = bass.AP(eb.tensor, eb.offset + 27,
                             [pstep, [BW, G], [32, 4], [1, 5]])
            nc.vector.memset(pad_ap, 0.0)

        zb = zsb[:]
        zsb3 = zb.rearrange("p (g c) -> p g c", c=BW)

        def phase_a_chunk(n, c, eb, pstep, eall):
            # gather + transpose one 128-sample chunk into Eall, then
            # immediately run that chunk's 32 Gram groups (keeps the PE
            # fed and shortens the per-tile pipeline fill).
            C = CH * n + c
            idxt = gp.tile([128, NE], _dt_i32, name="idxt")
            nc.sync.dma_start(idxt[:], t["idx"][128 * C:128 * (C + 1), :])
            esm = gp.tile([128, NE * D], _dt_bf16, name="esm")
            nc.gpsimd.indirect_dma_start(
                out=esm[:], out_offset=None,
                in_=t["tbl"][:],
                in_offset=bass.IndirectOffsetOnAxis(ap=idxt[:], axis=0),
            )
            for t8 in range(7):
                nt8 = 4 if t8 < 6 else 2
                trp = trps.tile([128, 128 * nt8], _dt_f32,
                                name="trp", tag="trp")
                for k in range(nt8):
                    ti = 4 * t8 + k
                    # out = esm_slice.T @ I : the stationary load is
                    # the transpose; FWL applies (128-col bf16).
                    nc.tensor.matmul(
                        trp[:, 128 * k:128 * (k + 1)],
                        esm[:, 128 * ti:128 * (ti + 1)], ident[:],
                        start=True, stop=True)
                tb = trp[:]
                # iterate (group, s, slot) so the nt8 adjacent slots are
                # innermost: dst runs of nt8 contiguous bf16 elements
                # instead of isolated 2-byte writes.
                srcv = bass.AP(tb.tensor, tb.offset,
                               [tb.ap[0], [4, 32], [1, 4], [128, nt8]])
                dst = bass.AP(
                    eb.tensor, eb.offset + BW * 32 * c + 4 * t8 + 1,
                    [pstep, [BW, 32], [32, 4], [1, nt8]])
                nc.vector.tensor_copy(dst, srcv)
            # Gram for this chunk's 32 groups
            for r in range(8):
                bank = grps.tile([128, 4 * BW], _dt_f32, name="grb", tag="gr")
                for k in range(4):
                    g = 32 * c + 4 * r + k
                    blk = eall[:, BW * g:BW * (g + 1)]
                    nc.tensor.matmul(bank[:, BW * k:BW * (k + 1)],
                                     blk, blk, start=True, stop=True)
                ro = 32 * c + 4 * r
                nc.scalar.activation(
                    zsb[:, BW * ro:BW * (ro + 4)], bank[:],
                    mybir.ActivationFunctionType.Copy)

        def phase_b(n, eb, pstep):
            # bottom MLP -> x into Eall slot 0
            h0 = hp.tile([128, 4 * 512], _dt_bf16, name="h0")
            for m in range(4):
                ps = mmps.tile([128, 512], _dt_f32, name="mm", tag="mm")
                nc.tensor.matmul(ps[:], bw0[:, 128 * m:128 * (m + 1)],
                                 dxt[:, TS * n:TS * (n + 1)],
                                 start=True, stop=True)
                nc.scalar.activation(h0[:, 512 * m:512 * (m + 1)], ps[:],
                                     Relu, bias=bb0[:, m:m + 1])
            h1b = hp.tile([128, 2 * 512], _dt_bf16, name="h1b")
            for m in range(2):
                ps = mmps.tile([128, 512], _dt_f32, name="mm", tag="mm")
                for k in range(4):
                    nc.tensor.matmul(
                        ps[:], bw1[:, 256 * k + 128 * m:256 * k + 128 * (m + 1)],
                        h0[:, 512 * k:512 * (k + 1)],
                        start=(k == 0), stop=(k == 3))
                nc.scalar.activation(h1b[:, 512 * m:512 * (m + 1)], ps[:],
                                     Relu, bias=bb1[:, m:m + 1])
            ps = mmps.tile([128, 512], _dt_f32, name="mm", tag="mm")
            for k in range(2):
                nc.tensor.matmul(ps[:], bw2[:, 128 * k:128 * (k + 1)],
                                 h1b[:, 512 * k:512 * (k + 1)],
                                 start=(k == 0), stop=(k == 1))
            xdst = bass.AP(eb.tensor, eb.offset, [pstep, [BW, G], [32, 4]])
            nc.scalar.activation(xdst, ps[:], Relu, bias=bb2[:, 0:1])

        def phase_d(n):
            # scatter lower-triangle Z rows into 32-aligned strips.
            # zpk columns are s-major (col = 128*s + g  <->  sample 4g+s)
            # so each copy writes one contiguous 128-column run; odd rows
            # go to the scalar engine to split the load with the DVE.
            Copy = mybir.ActivationFunctionType.Copy
            for i in range(1, NT):
                q, u = i // 4, i % 4
                p0 = 32 * u
                for s in range(4):
                    src = zsb3[32 * s:32 * s + i, :, 32 * s + i]
                    dst = zpk[q][p0:p0 + i, 128 * s:128 * (s + 1)]
                    # all scatters on the DVE: the scalar engine is the
                    # busiest queue (activations + Gram drains) and the
                    # top MLP of tile n blocks on these copies.
                    nc.vector.tensor_copy(dst, src)

        def phase_e(n, eb, pstep):
            # top MLP (layer 0: K = 128 x + 384 packed Z = 512)
            # s-major stream: element 128*s + g reads col 32*s + 128*g,
            # i.e. sample 4g+s -- matches the zpk column layout.
            xap = bass.AP(eb.tensor, eb.offset, [pstep, [32, 4], [BW, G]])
            h1t = hp.tile([128, 8 * 512], _dt_bf16, name="h1t")
            for m in range(8):
                ps = mmps.tile([128, 512], _dt_f32, name="mm", tag="mm")
                nc.tensor.matmul(ps[:], w0x[:, 128 * m:128 * (m + 1)], xap,
                                 start=True, stop=False)
                for q in range(NZK):
                    nc.tensor.matmul(
                        ps[:],
                        wzd[:, 1024 * q + 128 * m:1024 * q + 128 * (m + 1)],
                        zpk[q][:], start=False, stop=(q == NZK - 1))
                nc.scalar.activation(h1t[:, 512 * m:512 * (m + 1)], ps[:],
                                     Relu, bias=tb0[:, m:m + 1])
            h2t = hp.tile([128, 8 * 512], _dt_bf16, name="h2t")
            for m in range(8):
                ps = mmps.tile([128, 512], _dt_f32, name="mm", tag="mm")
                for k in range(8):
                    nc.tensor.matmul(
                        ps[:], w1[:, 1024 * k + 128 * m:1024 * k + 128 * (m + 1)],
                        h1t[:, 512 * k:512 * (k + 1)],
                        start=(k == 0), stop=(k == 7))
                nc.scalar.activation(h2t[:, 512 * m:512 * (m + 1)], ps[:],
                                     Relu, bias=tb1[:, m:m + 1])
            h3t = hp.tile([128, 4 * 512], _dt_bf16, name="h3t")
            for m in range(4):
                ps = mmps.tile([128, 512], _dt_f32, name="mm", tag="mm")
                for k in range(8):
                    nc.tensor.matmul(
                        ps[:], w2[:, 512 * k + 128 * m:512 * k + 128 * (m + 1)],
                        h2t[:, 512 * k:512 * (k + 1)],
                        start=(k == 0), stop=(k == 7))
                nc.scalar.activation(h3t[:, 512 * m:512 * (m + 1)], ps[:],
                                     Relu, bias=tb2[:, m:m + 1])
            ps3 = w3ps.tile([1, 512], _dt_f32, name="w3p", tag="w3")
            for k in range(4):
                nc.tensor.matmul(ps3[:], w3[:, k:k + 1],
                                 h3t[:, 512 * k:512 * (k + 1)],
                                 start=(k == 0), stop=(k == 3))
            outsb = op.tile([1, 512], _dt_f32, name="outsb")
            nc.scalar.activation(outsb[:], ps3[:], Sigmoid, bias=tb3[0:1, 0:1])
            nc.sync.dma_start(t["out"][n:n + 1, :], outsb[:])

        # software-pipelined emission: tile n's top MLP is emitted after
        # tile n+1's gather/bottom/Gram so the PE has work while the DVE
        # scatters tile n's Z. zpk is single-buffered: D(n) is emitted
        # after E(n-1) (program order covers the WAR hazard).
        for n in range(NTILES):
            eall = ealls[n % 2]
            eb = eall[:]
            pstep = eb.ap[0]
            phase_b(n, eb, pstep)
            for c in range(CH):
                phase_a_chunk(n, c, eb, pstep, eall)
            if n > 0:
                ep = ealls[(n - 1) % 2][:]
                phase_e(n - 1, ep, ep.ap[0])
            phase_d(n)
        ep = ealls[(NTILES - 1) % 2][:]
        phase_e(NTILES - 1, ep, ep.ap[0])


def _build():
    if "nc" in _CACHE:
        return _CACHE["nc"]
    nc = bacc.Bacc("TRN2", target_bir_lowering=False, debug=False,
                   num_devices=N_CORES)
    t = {}

    def dram(name, shape, dt, kind="ExternalInput"):
        t[name] = nc.dram_tensor(name, shape, dt, kind=kind).ap()

    dram("tbl", [NE * VOCAB, D], _dt_bf16)
    dram("idx", [SPC, NE], _dt_i32)
    dram("dxt", [16, SPC], _dt_bf16)
    dram("bw0", [16, 512], _dt_bf16)
    dram("bb0", [128, 4], _dt_f32)
    dram("bw1", [128, 4 * 256], _dt_bf16)
    dram("bb1", [128, 2], _dt_f32)
    dram("bw2", [128, 2 * 128], _dt_bf16)
    dram("bb2", [128, 1], _dt_f32)
    dram("w0x", [128, 1024], _dt_bf16)
    dram("wzd", [128, NZK * 1024], _dt_bf16)
    dram("tb0", [128, 8], _dt_f32)
    dram("w1", [128, 8 * 1024], _dt_bf16)
    dram("tb1", [128, 8], _dt_f32)
    dram("w2", [128, 8 * 512], _dt_bf16)
    dram("tb2", [128, 4], _dt_f32)
    dram("w3", [128, 4], _dt_bf16)
    dram("tb3", [1, 1], _dt_f32)
    dram("out", [NTILES, TS], _dt_f32, kind="ExternalOutput")

    with tile.TileContext(nc) as tc:
        _emit(tc, t)
    nc.compile()

    _CACHE["nc"] = nc
    return nc


def _ktile(w, kt, m):
    """[K, M] -> [128, (K//128) * M] with column kt*M + mm = w[128*kt + p, mm]."""
    K, Mo = w.shape
    return np.ascontiguousarray(
        w.reshape(K // 128, 128, Mo).transpose(1, 0, 2).reshape(128, -1))


def _shared_inputs(inputs):
    emb = np.asarray(inputs["emb_tables"])
    tbl = np.ascontiguousarray(
        emb.astype(BF16).reshape(NE * VOCAB, D))

    sh = {"tbl": tbl}
    sh["bw0"] = np.zeros((16, 512), BF16)
    sh["bw0"][:13] = np.asarray(inputs["bot_W0"]).astype(BF16)
    sh["bb0"] = np.asarray(inputs["bot_b0"]).astype(F32).reshape(4, 128).T.copy()
    sh["bw1"] = _ktile(np.asarray(inputs["bot_W1"]).astype(BF16), 4, 256)
    sh["bb1"] = np.asarray(inputs["bot_b1"]).astype(F32).reshape(2, 128).T.copy()
    sh["bw2"] = _ktile(np.asarray(inputs["bot_W2"]).astype(BF16), 2, 128)
    sh["bb2"] = np.asarray(inputs["bot_b2"]).astype(F32).reshape(1, 128).T.copy()

    w0 = np.asarray(inputs["top_W0"]).astype(F32)
    sh["w0x"] = w0[:128].astype(BF16)
    wzd_full = np.zeros((NZK * 128, 1024), F32)
    for f, (i, j) in enumerate(zip(LI, LJ)):
        q, u = i // 4, i % 4
        wzd_full[128 * q + 32 * u + j] = w0[128 + f]
    sh["wzd"] = _ktile(wzd_full, NZK, 1024).astype(BF16)
    sh["tb0"] = np.asarray(inputs["top_b0"]).astype(F32).reshape(8, 128).T.copy()
    sh["w1"] = _ktile(np.asarray(inputs["top_W1"]).astype(BF16), 8, 1024)
    sh["tb1"] = np.asarray(inputs["top_b1"]).astype(F32).reshape(8, 128).T.copy()
    sh["w2"] = _ktile(np.asarray(inputs["top_W2"]).astype(BF16), 8, 512)
    sh["tb2"] = np.asarray(inputs["top_b2"]).astype(F32).reshape(4, 128).T.copy()
    sh["w3"] = _ktile(np.asarray(inputs["top_W3"]).astype(BF16), 4, 1)
    sh["tb3"] = np.asarray(inputs["top_b3"]).astype(F32).reshape(1, 1)
    return sh


def _in_maps(inputs):
    sh = _shared_inputs(inputs)
    idx = np.asarray(inputs["indices"]).astype(np.int64)      # [26, B]
    gidx = (idx + (np.arange(NE) * VOCAB)[:, None]).astype(np.int32)
    dx = np.asarray(inputs["dense_x"]).astype(F32)            # [B, 13]
    maps = []
    for core in range(N_CORES):
        sl = slice(SPC * core, SPC * (core + 1))
        m = dict(sh)
        m["idx"] = np.ascontiguousarray(gidx[:, sl].T)        # [2048, 26]
        dxt = np.zeros((16, SPC), BF16)
        dxt[:13] = dx[sl].T.astype(BF16)
        m["dxt"] = dxt
        maps.append(m)
    return maps


def _run(inputs, trace=False):
    nc = _build()
    maps = _in_maps(inputs)
    old_m = nc.m
    nc.m = _CACHE.setdefault("hwm", get_hw_module(nc.m))
    try:
        res = bass_utils.run_bass_kernel_spmd(
            nc, maps, core_ids=list(range(N_CORES)), trace=trace)
    finally:
        nc.m = old_m
    outs = []
    for r in res.results:
        o = r["out"].reshape(NTILES, 4, 128)      # [n, s, g]
        outs.append(o.transpose(0, 2, 1).reshape(-1))  # sample 4g+s
    out = np.concatenate(outs)
    return out.astype(F32).reshape(B, 1), res


def kernel(**inputs):
    out, _ = _run(inputs, trace=False)
    return out
